# revision 1
# baseline (speedup 1.0000x reference)
"""Trainium2 Bass kernel for causal multi-head attention (prefill).

Problem: x[2,2048,768], 12 heads x 64 dim, causal softmax(QK^T/8)V + out-proj.

Sharding (8 cores, no collectives): core c handles batch c//4 and head group
c%4 (3 heads).  Each core computes, for its batch b and heads hs:
    qT,kT = (Wq_hs @ x_b^T), (Wk_hs @ x_b^T)        [192, 2048] (transposed)
    v     = x_b @ Wv_hs^T                            [2048, 192+ones]
    expT  = exp(scoresT/8) masked causally           [kv, sq] per head
    ctxT_h = v_aug^T @ expT  (extra row = softmax denom via ones column)
    outT_partial = Wo[:,cols_hs] @ (ctxT/den)        [768, 2048]
Host sums the 4 partial outputs per batch and transposes back.

All matmuls run as float32r (full-rate fp32 on the PE at N>=256); every
tensor feeding a matmul is float32r end-to-end (walrus requires producers
to round to f32r).  Softmax skips the max-subtraction: scores/8 ~ N(0,1),
so exp stays in fp32 range.  Causal masking: below-diagonal blocks are
computed at partial width starting at the diagonal, the 128x128 diagonal
triangle is masked by multiplying with a host-provided 0/1 mask, and
above-diagonal regions are simply never computed nor read.
"""

import numpy as np

import concourse.bass as bass
import concourse.tile as tile
from concourse import bacc, mybir
from concourse.bass_utils import run_bass_kernel_spmd

F32 = mybir.dt.float32
F32R = mybir.dt.float32r

B, S, D = 2, 2048, 768
H, DH = 12, 64
HPC = 3                 # heads per core
GH = HPC * DH           # 192 head dims per core
NCORES = 8
KT = D // 128           # 6 contraction tiles for projections
NSQ = S // 512          # 4 sq blocks of 512
NKV = S // 128          # 16 kv tiles of 128
WJ = 1024               # exp/ctx window width
NJ = S // WJ            # 2 windows


def build():
    nc = bacc.Bacc("TRN2", target_bir_lowering=False, debug=False)

    xT = nc.dram_tensor("xT", [D, S], F32R, kind="ExternalInput")
    wq = nc.dram_tensor("wq", [D, GH], F32R, kind="ExternalInput")
    wk = nc.dram_tensor("wk", [D, GH], F32R, kind="ExternalInput")
    wv = nc.dram_tensor("wv", [D, 256], F32R, kind="ExternalInput")  # 192 + 64 pad
    wo = nc.dram_tensor("wo", [GH, D], F32R, kind="ExternalInput")
    tri = nc.dram_tensor("tri", [128, 128], F32R, kind="ExternalInput")
    onesd = nc.dram_tensor("onesd", [1, 64], F32R, kind="ExternalInput")
    outT = nc.dram_tensor("outT", [D, S], F32, kind="ExternalOutput")

    with tile.TileContext(nc) as tc, \
         nc.allow_low_precision(reason="fp32r tiles feeding fp32r matmuls"):
        with tc.tile_pool(name="sb", bufs=1) as sb, \
             tc.tile_pool(name="sbe", bufs=3) as sbe, \
             tc.tile_pool(name="sbo", bufs=2) as sbo, \
             tc.tile_pool(name="ps", bufs=2, space="PSUM") as ps, \
             tc.tile_pool(name="psc", bufs=1, space="PSUM") as psc:

            # ---- phase 0: load weights + x ----
            xsb = sb.tile([128, KT, S], F32R, tag="xsb")
            x_r = xT[:, :].rearrange("(k p) n -> p k n", p=128)
            for k in range(KT):
                nc.sync.dma_start(xsb[:, k, :], x_r[:, k, :])

            wq_sb = sb.tile([128, KT, GH], F32R, tag="wq")
            wk_sb = sb.tile([128, KT, GH], F32R, tag="wk")
            wv_sb = sb.tile([128, KT, 256], F32R, tag="wv")
            nc.sync.dma_start(wq_sb, wq[:, :].rearrange("(k p) m -> p k m", p=128))
            nc.sync.dma_start(wk_sb, wk[:, :].rearrange("(k p) m -> p k m", p=128))
            nc.sync.dma_start(wv_sb, wv[:, :].rearrange("(k p) m -> p k m", p=128))
            wo01_sb = sb.tile([128, D], F32R, tag="wo01")
            wo2_sb = sb.tile([64, D], F32R, tag="wo2")
            nc.sync.dma_start(wo01_sb, wo[0:128, :])
            nc.sync.dma_start(wo2_sb, wo[128:GH, :])
            tri_sb = sb.tile([128, 128], F32R, tag="tri")
            nc.sync.dma_start(tri_sb, tri[:, :])
            ones_sb = sb.tile([1, 64], F32R, tag="ones")
            nc.sync.dma_start(ones_sb, onesd[:, :])

            # ---- phase 1: projections ----
            # qT/kT: [192, S] as [128, 2, S] tiles (Mt0 = heads 0/1, Mt1 = head 2)
            qt_sb = sb.tile([128, 2, S], F32R, tag="qt")
            kt_sb = sb.tile([128, 2, S], F32R, tag="kt")
            for dst, wsb in ((qt_sb, wq_sb), (kt_sb, wk_sb)):
                for mt in range(2):          # 128 rows, then 64 rows
                    mp = 128 if mt == 0 else 64
                    for nt in range(NSQ):
                        pp = ps.tile([128, 512], F32, tag="sc", name="pp")
                        for k in range(KT):
                            nc.tensor.matmul(
                                pp[:mp, :],
                                wsb[:, k, mt * 128:mt * 128 + mp],
                                xsb[:, k, nt * 512:(nt + 1) * 512],
                                start=(k == 0), stop=(k == KT - 1))
                        nc.vector.tensor_copy(
                            dst[:mp, mt, nt * 512:(nt + 1) * 512], pp[:mp, :])

            # v_aug: [128, NKV, 195]; per kv tile: head h v at cols 65h..65h+63,
            # ones at col 65h+64 (written via ACT copy: tri*0 + 1)
            vaug = sb.tile([128, NKV, 195], F32R, tag="vaug")
            for h in range(HPC):
                nc.scalar.activation(
                    vaug[:, :, 65 * h + 64:65 * h + 65],
                    tri_sb[:, h * NKV:(h + 1) * NKV].rearrange(
                        "p (t c) -> p t c", c=1),
                    mybir.ActivationFunctionType.Copy, bias=1.0, scale=0.0)
            for i in range(NKV):
                pp = ps.tile([128, 256], F32, tag="sc", name="pp")
                for k in range(KT):
                    nc.tensor.matmul(
                        pp,
                        xsb[:, k, i * 128:(i + 1) * 128],
                        wv_sb[:, k, :],
                        start=(k == 0), stop=(k == KT - 1))
                nc.vector.tensor_copy(
                    vaug[:, i, :].rearrange("p (h c) -> p h c", c=65)[:, :, 0:64],
                    pp[:, 0:192].rearrange("p (h c) -> p h c", c=64))

            # ---- phase 2: attention ----
            # ctxT: heads 0/1 packed in one [128, S] tile (h1 via partition-
            # shifting sbuf-to-sbuf DMA), head 2 in its own [64, S] tile.
            ctxT01 = sb.tile([128, S], F32R, tag="ctxT01")
            ctxT2 = sb.tile([64, S], F32R, tag="ctxT2")
            for J in range(NJ):
                for h in range(HPC):
                    if h < 2:
                        def kslc(i, h=h):
                            return kt_sb[64 * h:64 * h + 64, 0, i * 128:(i + 1) * 128]

                        def qslc(c0, c1, h=h):
                            return qt_sb[64 * h:64 * h + 64, 0, c0:c1]
                    else:
                        def kslc(i):
                            return kt_sb[0:64, 1, i * 128:(i + 1) * 128]

                        def qslc(c0, c1):
                            return qt_sb[0:64, 1, c0:c1]

                    ctx_ps = psc.tile([65, WJ], F32, tag="ctx", name="ctx_ps")
                    imax = 8 * J + 7
                    for i in range(imax + 1):
                        d = 128 * i - WJ * J       # window col of diagonal start
                        col0 = max(0, d)
                        nb0 = max(0, d // 512)
                        spsum = ps.tile([128, WJ], F32, tag="sc", name="spsum")
                        for nb in range(nb0, 2):
                            s0 = max(nb * 512, col0)
                            nc.tensor.matmul(
                                spsum[:, s0:(nb + 1) * 512],
                                kslc(i),
                                qslc(WJ * J + s0, WJ * J + (nb + 1) * 512),
                                start=True, stop=True)
                        esb = sbe.tile([128, WJ], F32R, tag="exp", name="esb")
                        nc.scalar.activation(
                            esb[:, col0:WJ], spsum[:, col0:WJ],
                            mybir.ActivationFunctionType.Exp, scale=0.125)
                        if d >= 0:
                            nc.vector.tensor_mul(
                                esb[:, d:d + 128], esb[:, d:d + 128], tri_sb)
                        for nb in range(nb0, 2):
                            s0 = max(nb * 512, col0)
                            nc.tensor.matmul(
                                ctx_ps[:, s0:(nb + 1) * 512],
                                vaug[:, i, 65 * h:65 * h + 65],
                                esb[:, s0:(nb + 1) * 512],
                                start=(i == 0), stop=(i == 8 * J + 4 * nb + 3))
                    # normalize rows 0:64 by row 64 (softmax denominator)
                    h1tmp = (sbo.tile([64, WJ], F32R, tag="h1tmp", name="h1tmp")
                             if h == 1 else None)
                    for nb in range(2):
                        c0 = WJ * J + nb * 512
                        inv = sbo.tile([1, 512], F32R, tag="inv", name="inv")
                        nc.vector.reciprocal(
                            inv, ctx_ps[64:65, nb * 512:(nb + 1) * 512])
                        bps = ps.tile([64, 512], F32, tag="sc", name="bps")
                        nc.tensor.matmul(bps, ones_sb, inv, start=True, stop=True)
                        bsb = sbo.tile([64, 512], F32, tag="bsb", name="bsb")
                        nc.vector.tensor_copy(bsb, bps)
                        if h == 0:
                            dst = ctxT01[0:64, c0:c0 + 512]
                        elif h == 1:
                            dst = h1tmp[:, nb * 512:(nb + 1) * 512]
                        else:
                            dst = ctxT2[:, c0:c0 + 512]
                        nc.vector.tensor_mul(
                            dst, ctx_ps[0:64, nb * 512:(nb + 1) * 512], bsb)
                    if h == 1:
                        # partition-shift h1's ctxT into rows 64:128
                        nc.sync.dma_start(
                            ctxT01[64:128, WJ * J:WJ * (J + 1)], h1tmp)

                # ---- phase 3: out-projection for this window ----
                for j in (2 * J, 2 * J + 1):
                    for mt in range(6):
                        ops = ps.tile([128, 512], F32, tag="sc", name="ops")
                        nc.tensor.matmul(
                            ops, wo01_sb[:, mt * 128:(mt + 1) * 128],
                            ctxT01[:, j * 512:(j + 1) * 512],
                            start=True, stop=False)
                        nc.tensor.matmul(
                            ops, wo2_sb[:, mt * 128:(mt + 1) * 128],
                            ctxT2[:, j * 512:(j + 1) * 512],
                            start=False, stop=True)
                        osb = sbo.tile([128, 512], F32, tag="osb", name="osb")
                        nc.vector.tensor_copy(osb, ops)
                        nc.sync.dma_start(
                            outT[mt * 128:(mt + 1) * 128, j * 512:(j + 1) * 512],
                            osb)

    nc.compile()
    return nc


def shard_inputs(x, Wq, Wk, Wv, Wo):
    x = np.asarray(x, np.float32)
    tri = np.triu(np.ones((128, 128), np.float32))
    ones = np.ones((1, 64), np.float32)
    in_maps = []
    for c in range(NCORES):
        b, g = c // 4, c % 4
        rs = slice(GH * g, GH * g + GH)
        wv_t = np.concatenate(
            [np.ascontiguousarray(np.asarray(Wv, np.float32)[rs].T),
             np.zeros((D, 64), np.float32)], axis=1)
        in_maps.append({
            "xT": np.ascontiguousarray(x[b].T),
            "wq": np.ascontiguousarray(np.asarray(Wq, np.float32)[rs].T),
            "wk": np.ascontiguousarray(np.asarray(Wk, np.float32)[rs].T),
            "wv": wv_t,
            "wo": np.ascontiguousarray(np.asarray(Wo, np.float32)[:, rs].T),
            "tri": tri,
            "onesd": ones,
        })
    return in_maps


def assemble(results, bo):
    out = np.zeros((B, S, D), np.float32)
    for c in range(NCORES):
        out[c // 4] += results[c]["outT"].T
    return out + np.asarray(bo, np.float32)[None, None, :]


_NC = None


def kernel(x, Wq, Wk, Wv, Wo, bo, **run_kwargs):
    global _NC
    if _NC is None:
        _NC = build()
    in_maps = shard_inputs(x, Wq, Wk, Wv, Wo)
    res = run_bass_kernel_spmd(_NC, in_maps, core_ids=list(range(NCORES)),
                               **run_kwargs)
    out = assemble(res.results, bo)
    kernel.last_results = res
    return out



# revision 7
# speedup vs baseline: 1.8823x; 1.8823x over previous
"""Trainium2 Bass kernel for causal multi-head attention (prefill), v2.

Problem: x[2,2048,768], 12 heads x 64 dim, causal softmax(QK^T/8)V + out-proj.

Sharding (8 cores, no collectives): core c handles batch c//4 and head group
c%4 (3 heads).  Host sums the 4 partial outputs per batch and transposes.

v2 vs v1 (290us): everything bf16 (half DMA, FWL weight loads, smaller SBUF),
packed Q+K projection (3 full 128-row tiles instead of 4), v-projection at
N=192 (no f32r N>=256 constraint), softmax denominator replicated to 64 PSUM
rows for free via ones-columns in the ctx matmul weights, normalization via
the fast custom-DVE reciprocal (replaces 40us of single-lane DVE reciprocal +
PE broadcast matmuls), x loaded via 12 parallel DMA queues, attention emitted
as early as its projection deps allow so PE/ACT overlap across phases.

Per core, for its batch b and heads hs (3 heads):
    qkT  = [Wq_hs; Wk_hs] @ x_b^T            [384, 2048]  (3 x 128-row tiles)
    vaug = per (kv-tile i, head h): [v_h_i | ones]   [128, 128] ctx weights
    per (J window of 1024 q, head h, kv tile i <= diag):
       scores = k_h_i^T q_h  -> exp(s/8) -> masked on diagonal tile
       ctx_ps[0:64]   += v_h_i^T  exp      (head dims)
       ctx_ps[64:128] += ones^T   exp      (= softmax denom, replicated)
    ctxn_h = ctx_ps[0:64] * recip_approx(ctx_ps[64:128])
    outT_partial = [Wo cols]^T @ ctxn       [768, 2048]
"""

import numpy as np

import concourse.bass as bass
import concourse.tile as tile
from concourse import bacc, mybir
from concourse.bass_utils import run_bass_kernel_spmd

F32 = mybir.dt.float32
BF = mybir.dt.bfloat16

B, S, D = 2, 2048, 768
H, DH = 12, 64
HPC = 3                 # heads per core
GH = HPC * DH           # 192 head dims per core
NCORES = 8
KT = D // 128           # 6 contraction tiles for projections
NKV = S // 128          # 16 kv tiles of 128
WJ = 1024               # attention q-window width
NJ = S // WJ            # 2 windows
NCH = 2                 # x / projection column chunks of 1024


def build():
    nc = bacc.Bacc("TRN2", target_bir_lowering=False, debug=False)

    xT = nc.dram_tensor("xT", [D, S], BF, kind="ExternalInput")
    wqk = nc.dram_tensor("wqk", [D, 2 * GH], BF, kind="ExternalInput")
    wv = nc.dram_tensor("wv", [D, GH], BF, kind="ExternalInput")
    wo = nc.dram_tensor("wo", [GH, D], BF, kind="ExternalInput")
    tri = nc.dram_tensor("tri", [128, 128], BF, kind="ExternalInput")
    outT = nc.dram_tensor("outT", [D, S], BF, kind="ExternalOutput")

    with tile.TileContext(nc) as tc, \
         nc.allow_low_precision(reason="bf16 compute, fp32 accumulation"):
        with tc.tile_pool(name="sb", bufs=1) as sb, \
             tc.tile_pool(name="sbe", bufs=3) as sbe, \
             tc.tile_pool(name="sbo", bufs=2) as sbo, \
             tc.tile_pool(name="ps", bufs=2, space="PSUM") as ps, \
             tc.tile_pool(name="psc", bufs=2, space="PSUM") as psc:

            # ---- phase 0: loads ----
            wqk_sb = sb.tile([128, KT, 2 * GH], BF, tag="wqk")
            wqk_r = wqk[:, :].rearrange("(k p) m -> p k m", p=128)
            for m in range(3):
                nc.sync.dma_start(wqk_sb[:, :, m * 128:(m + 1) * 128],
                                  wqk_r[:, :, m * 128:(m + 1) * 128])
            wv_sb = sb.tile([128, KT, GH], BF, tag="wv")
            nc.sync.dma_start(wv_sb, wv[:, :].rearrange("(k p) m -> p k m", p=128))

            xsb = sb.tile([128, KT, S], BF, tag="xsb")
            x_r = xT[:, :].rearrange("(k p) n -> p k n", p=128)
            for ch in range(NCH):
                for k in range(KT):
                    nc.sync.dma_start(
                        xsb[:, k, ch * WJ:(ch + 1) * WJ],
                        x_r[:, k, ch * WJ:(ch + 1) * WJ])

            tri_sb = sb.tile([128, 128], BF, tag="tri")
            nc.sync.dma_start(tri_sb, tri[:, :])
            wo01_sb = sb.tile([128, D], BF, tag="wo01")
            wo2_sb = sb.tile([64, D], BF, tag="wo2")
            nc.sync.dma_start(wo01_sb, wo[0:128, :])
            nc.sync.dma_start(wo2_sb, wo[128:GH, :])

            # ---- persistent sbuf tensors ----
            # qkT packed rows: [q_h0|q_h1], [k_h0|k_h1], [q_h2|k_h2];
            # k_h2 is re-copied to partition base 0 (kh2b) so every head's
            # q/k pair shares a base partition (matmul requirement).
            qkT = sb.tile([128, 3, S], BF, tag="qkT")
            kh2b = sb.tile([64, S], BF, tag="kh2b")
            # ctx matmul weights: per (i, h): cols 0:64 = v_h_i, 64:128 = ones
            vaug = sb.tile([128, NKV, HPC, 128], BF, tag="vaug")
            nc.vector.memset(vaug[:, :, :, 64:128], 1.0)
            ctxT01 = sb.tile([128, S], BF, tag="ctxT01")
            ctxT2 = sb.tile([64, S], BF, tag="ctxT2")

            def qslc(h, c0, c1):
                return (qkT[0:64, 0, c0:c1], qkT[64:128, 0, c0:c1],
                        qkT[0:64, 2, c0:c1])[h]

            def kslc(h, i):
                c0, c1 = i * 128, (i + 1) * 128
                return (qkT[0:64, 1, c0:c1], qkT[64:128, 1, c0:c1],
                        kh2b[:, c0:c1])[h]

            def proj_qk(mt, ch):
                c0 = ch * WJ
                pp = ps.tile([128, WJ], F32, tag="sc", name="pp")
                for nb in range(2):
                    s0 = nb * 512
                    for k in range(KT):
                        nc.tensor.matmul(
                            pp[:, s0:s0 + 512],
                            wqk_sb[:, k, mt * 128:(mt + 1) * 128],
                            xsb[:, k, c0 + s0:c0 + s0 + 512],
                            start=(k == 0), stop=(k == KT - 1))
                if mt < 2:
                    nc.vector.tensor_copy(qkT[:, mt, c0:c0 + WJ], pp)
                else:
                    nc.vector.tensor_copy(qkT[0:64, 2, c0:c0 + WJ], pp[0:64, :])
                    nc.vector.tensor_copy(kh2b[:, c0:c0 + WJ], pp[64:128, :])

            def proj_v(i):
                pp = ps.tile([128, WJ], F32, tag="sc", name="pp")
                for k in range(KT):
                    nc.tensor.matmul(
                        pp[:, 0:GH],
                        xsb[:, k, i * 128:(i + 1) * 128],
                        wv_sb[:, k, :],
                        start=(k == 0), stop=(k == KT - 1))
                nc.vector.tensor_copy(
                    vaug[:, i, :, 0:64],
                    pp[:, 0:GH].rearrange("p (h c) -> p h c", c=64))

            def attention(J, h):
                ctx_ps = psc.tile([128, WJ], F32, tag="ctx", name="ctx_ps")
                imax = 8 * J + 7
                for i in range(imax + 1):
                    d = 128 * i - WJ * J   # window col where the diagonal starts
                    col0 = max(0, d)
                    nb0 = max(0, d // 512)
                    spsum = ps.tile([128, WJ], F32, tag="sc", name="spsum")
                    for nb in range(nb0, 2):
                        s0 = max(nb * 512, col0)
                        nc.tensor.matmul(
                            spsum[:, s0:(nb + 1) * 512],
                            kslc(h, i),
                            qslc(h, WJ * J + s0, WJ * J + (nb + 1) * 512),
                            start=True, stop=True)
                    esb = sbe.tile([128, WJ], BF, tag="exp", name="esb")
                    nc.scalar.activation(
                        esb[:, col0:WJ], spsum[:, col0:WJ],
                        mybir.ActivationFunctionType.Exp, scale=0.125)
                    if d >= 0:
                        nc.vector.tensor_mul(
                            esb[:, d:d + 128], esb[:, d:d + 128], tri_sb)
                    for nb in range(nb0, 2):
                        s0 = max(nb * 512, col0)
                        nc.tensor.matmul(
                            ctx_ps[:, s0:(nb + 1) * 512],
                            vaug[:, i, h, :],
                            esb[:, s0:(nb + 1) * 512],
                            start=(i == 0), stop=(i == 8 * J + 4 * nb + 3))
                # normalize: rows 64:128 hold the softmax denominator, replicated
                # (reciprocal_approx_fast can't read PSUM -> bounce via SBUF)
                den = sbo.tile([64, WJ], F32, tag="den", name="den")
                nc.vector.tensor_copy(den, ctx_ps[64:128, :])
                inv = sbo.tile([64, WJ], F32, tag="inv", name="inv")
                nc.vector.reciprocal_approx_fast(inv, den)
                h1tmp = (sbo.tile([64, WJ], BF, tag="h1tmp", name="h1tmp")
                         if h == 1 else None)
                dst = (ctxT01[0:64, WJ * J:WJ * (J + 1)], h1tmp,
                       ctxT2[:, WJ * J:WJ * (J + 1)])[h]
                nc.vector.tensor_mul(dst, ctx_ps[0:64, :], inv)
                if h == 1:
                    # partition-shift h1's ctx into ctxT01 rows 64:128
                    nc.sync.dma_start(
                        ctxT01[64:128, WJ * J:WJ * (J + 1)], h1tmp)

            def out_proj(J, mt):
                ops = psc.tile([128, WJ], F32, tag="ctx", name="ops")
                for jj in range(2):
                    c0 = WJ * J + jj * 512
                    nc.tensor.matmul(
                        ops[:, jj * 512:(jj + 1) * 512],
                        wo01_sb[:, mt * 128:(mt + 1) * 128],
                        ctxT01[:, c0:c0 + 512], start=True, stop=False)
                    nc.tensor.matmul(
                        ops[:, jj * 512:(jj + 1) * 512],
                        wo2_sb[:, mt * 128:(mt + 1) * 128],
                        ctxT2[:, c0:c0 + 512], start=False, stop=True)
                osb = sbo.tile([128, WJ], BF, tag="osb", name="osb")
                nc.vector.tensor_copy(osb, ops)
                nc.sync.dma_start(
                    outT[mt * 128:(mt + 1) * 128, WJ * J:WJ * (J + 1)], osb)

            # ---- emission order == scheduler priority ----
            # J0 attention needs: qk mt0 (q h0/h1), mt1 (q h2, k h0) on ch0,
            # and vaug i 0..7; emit those first, then interleave the rest.
            proj_qk(0, 0)
            proj_qk(1, 0)
            for i in range(8):
                proj_v(i)
            attention(0, 0)
            proj_qk(2, 0)
            for i in range(8, NKV):
                proj_v(i)
            attention(0, 1)
            for mt in range(3):
                proj_qk(mt, 1)
            attention(0, 2)
            for mt in range(6):
                out_proj(0, mt)
            for h in range(HPC):
                attention(1, h)
            for mt in range(6):
                out_proj(1, mt)

    nc.compile()
    return nc


def shard_inputs(x, Wq, Wk, Wv, Wo):
    x = np.asarray(x, np.float32)
    tri = np.triu(np.ones((128, 128), np.float32)).astype(np.float32)

    def bf(a):
        import ml_dtypes
        return np.ascontiguousarray(a).astype(ml_dtypes.bfloat16)

    in_maps = []
    for c in range(NCORES):
        b, g = c // 4, c % 4
        rs = slice(GH * g, GH * g + GH)
        Wq_g = np.asarray(Wq, np.float32)[rs]  # [192, 768]
        Wk_g = np.asarray(Wk, np.float32)[rs]
        # packed rows: [q0 q1 | k0 k1 | q2 k2] (64 rows each)
        wqk = np.concatenate(
            [Wq_g[0:128], Wk_g[0:128], Wq_g[128:192], Wk_g[128:192]],
            axis=0).T  # [768, 384]
        in_maps.append({
            "xT": bf(x[b].T),
            "wqk": bf(wqk),
            "wv": bf(np.asarray(Wv, np.float32)[rs].T),
            "wo": bf(np.asarray(Wo, np.float32)[:, rs].T),
            "tri": bf(tri),
        })
    return in_maps


def assemble(results, bo):
    out = np.zeros((B, S, D), np.float32)
    for c in range(NCORES):
        out[c // 4] += results[c]["outT"].astype(np.float32).T
    return out + np.asarray(bo, np.float32)[None, None, :]


_NC = None


def kernel(x, Wq, Wk, Wv, Wo, bo, **run_kwargs):
    global _NC
    if _NC is None:
        _NC = build()
    in_maps = shard_inputs(x, Wq, Wk, Wv, Wo)
    res = run_bass_kernel_spmd(_NC, in_maps, core_ids=list(range(NCORES)),
                               **run_kwargs)
    out = assemble(res.results, bo)
    kernel.last_results = res
    return out
